# revision 20
# baseline (speedup 1.0000x reference)
"""GAT message-passing kernel for Trainium2 (8 NeuronCores, Bass/Tile).

Strategy (edge-parallel, dst-block partitioning): the model output
y = elu(sum(xo[0] * xo[1:item_len], 1)) depends only on output rows
0..item_len-1, so only edges with dst < item_len contribute (~33K of
3.2M edges). Core k owns dst rows [128k, 128k+128); every core also
processes the dst==0 edges so xo[0] is available locally.

Gather: per-128-row indirect DMA on the GpSimd SWDGE queue costs
~994 ns fixed + ~0.7 ns/row (measured 1090 ns/tile + ~309 ns dispatch
gap) — the 34-tile stream (~47 us) is the kernel's critical path.
The SWDGE ucode honors only 128 offsets per instruction (extra index
columns are misread as a larger element size), so batching beyond 128
rows/instruction is not possible; everything else is arranged to hide
under the stream:

  tensor:  ad_e column = S_t^T @ a_d        (host-shipped one-hot S)
  dve/act: xbf = bf16(x_src)                (cast, engines alternated)
  dve:     a_s = rowsum(xbf * w_s)          (bf16 stt with accumulate)
  batched: p = exp(leaky(a_s + a_d))        (per tile-group)
  dve:     S_p = p * S_t                    (scale one-hot rows by p)
  tensor:  acc += S_p^T @ [xbf | 1]         (bf16 matmul, f32 PSUM)

Scaling the 128-wide one-hot block by p (instead of scaling the
256-wide gathered row) plus a constant ones-column for z cuts per-tile
DVE/ACT work ~2x, so compute never throttles the gather stream (DVE
port pressure also slows the Q7 descriptor writes). The group pipeline
is software-skewed with tapered final groups; the main accumulator is
split four ways so 3/4 of the (u@W) epilogue runs mid-stream.
out = (u@W)/z + bias; xo = elu(out); y = elu(xo_m @ xo[0]). The host
precomputes W@att_src / W@att_dst, casts W to bf16, and builds the
one-hot S / S^T blocks (index metadata only; x is never touched beyond
contiguous slicing).
"""
import math

import numpy as np
import ml_dtypes

P = 128
N_CORES = 8
NEG_SLOPE = 0.2
IND = 256
OUTD = 128
W258 = IND + 2  # xbf tile stride: 256 data + 1 ones column + 1 pad
BF16 = ml_dtypes.bfloat16
KGRP = 4  # tiles per compute group (small-op batching)

_CACHE = {}


def _build_program(n_nodes, T_d):
    import concourse.bass as bass
    import concourse.bacc as bacc
    import concourse.tile as tile
    import concourse.mybir as mybir
    from contextlib import ExitStack

    f32 = mybir.dt.float32
    bf16 = mybir.dt.bfloat16
    i32 = mybir.dt.int32
    Alu = mybir.AluOpType
    Act = mybir.ActivationFunctionType

    T_all = T_d + 1  # data tiles + self tile (dst0 edges ride tile 0's pad)
    T_idx = max(T_d, 128)  # pad idx lines to 512B: sub-512B DMA pays RMW + slow receipt

    nc = bacc.Bacc(
        "TRN2", target_bir_lowering=False, debug=False, num_devices=N_CORES
    )
    x_in = nc.dram_tensor("x_in", [n_nodes, IND], f32, kind="ExternalInput").ap()
    idx_in = nc.dram_tensor("idx_in", [P, T_idx], i32, kind="ExternalInput").ap()
    xself_in = nc.dram_tensor("xself_in", [P, IND], f32, kind="ExternalInput").ap()
    x0_in = nc.dram_tensor("x0_in", [1, IND], f32, kind="ExternalInput").ap()
    s_in = nc.dram_tensor("s_in", [P, T_all * P], bf16, kind="ExternalInput").ap()
    st_in = nc.dram_tensor("st_in", [P, T_all * P], bf16, kind="ExternalInput").ap()
    ws_in = nc.dram_tensor("ws_in", [1, IND], f32, kind="ExternalInput").ap()
    wd_in = nc.dram_tensor("wd_in", [1, IND], f32, kind="ExternalInput").ap()
    w_in = nc.dram_tensor("w_in", [IND, OUTD], bf16, kind="ExternalInput").ap()
    bias_in = nc.dram_tensor("bias_in", [1, OUTD], f32, kind="ExternalInput").ap()
    ident_in = nc.dram_tensor("ident_in", [P, P], bf16, kind="ExternalInput").ap()
    sb_in = nc.dram_tensor("sb_in", [P, P], bf16, kind="ExternalInput").ap()
    mask0_in = nc.dram_tensor("mask0_in", [P, 1], f32, kind="ExternalInput").ap()
    y_out = nc.dram_tensor("y_out", [1, P], f32, kind="ExternalOutput").ap()

    with tile.TileContext(nc) as tc, ExitStack() as ctx:
        const = ctx.enter_context(tc.tile_pool(name="const", bufs=1))
        idxp = ctx.enter_context(tc.tile_pool(name="idx", bufs=1))
        xgp = ctx.enter_context(tc.tile_pool(name="xg", bufs=T_d))
        xbfp = ctx.enter_context(tc.tile_pool(name="xbf", bufs=KGRP + 3))
        spp = ctx.enter_context(tc.tile_pool(name="sp", bufs=KGRP + 3))
        scrp = ctx.enter_context(tc.tile_pool(name="scr", bufs=3))
        grpp = ctx.enter_context(tc.tile_pool(name="grp", bufs=12))
        smallp = ctx.enter_context(tc.tile_pool(name="small", bufs=10))
        utp = ctx.enter_context(tc.tile_pool(name="ut", bufs=2))
        xotr = ctx.enter_context(tc.tile_pool(name="xotr", bufs=4))
        xop = ctx.enter_context(tc.tile_pool(name="xo", bufs=2))
        accsb = ctx.enter_context(tc.tile_pool(name="accsb", bufs=2))
        # PSUM banks: acc_b 1 + acc_m 3 + tp 1 + outp 2 + adp 1 = 8
        accp = ctx.enter_context(tc.tile_pool(name="acc", bufs=4, space="PSUM"))
        tpp = ctx.enter_context(tc.tile_pool(name="tp", bufs=1, space="PSUM"))
        outpp = ctx.enter_context(tc.tile_pool(name="outp", bufs=1, space="PSUM"))
        adpp = ctx.enter_context(tc.tile_pool(name="adp", bufs=1, space="PSUM"))

        # ---- index DMA first: the gather stream depends only on this ----
        idx_t = idxp.tile([P, T_idx], i32, tag="idx")
        nc.sync.dma_start(idx_t[:], idx_in[:])

        # data tiles: one indirect DMA per 128 rows (tile 0's tail slots
        # carry the dst0 edges shared by all cores)
        xg_d = []
        for t in range(T_d):
            xg = xgp.tile([P, IND], f32, tag="xg")
            nc.gpsimd.indirect_dma_start(
                out=xg[:],
                out_offset=None,
                in_=x_in[:],
                in_offset=bass.IndirectOffsetOnAxis(
                    ap=idx_t[:, t : t + 1], axis=0
                ),
            )
            xg_d.append(xg)

        # ---- remaining input DMAs (small + early-needed first) ----
        xself_t = const.tile([P, IND], f32, tag="xself")
        nc.sync.dma_start(xself_t[:], xself_in[:])
        x0_t = const.tile([1, IND], f32, tag="x0")
        nc.sync.dma_start(x0_t[:], x0_in[:])
        ws_t = const.tile([1, IND], f32, tag="ws_t")
        nc.sync.dma_start(ws_t[:], ws_in[:])
        wd_t = const.tile([1, IND], f32, tag="wd_t")
        nc.sync.dma_start(wd_t[:], wd_in[:])
        # st needed first (stage-A ad matmuls), s shortly after; parallel queues
        st_t = const.tile([P, T_all * P], bf16, tag="st_t")
        nc.sync.dma_start(st_t[:], st_in[:])
        s_t = const.tile([P, T_all * P], bf16, tag="s_t")
        nc.scalar.dma_start(s_t[:], s_in[:])
        W0 = const.tile([P, OUTD], bf16, tag="W0")
        nc.scalar.dma_start(W0[:], w_in[0:P, :])
        W1 = const.tile([P, OUTD], bf16, tag="W1")
        nc.scalar.dma_start(W1[:], w_in[P : 2 * P, :])
        bias_t = const.tile([1, OUTD], f32, tag="bias")
        nc.scalar.dma_start(bias_t[:], bias_in[:])
        ident = const.tile([P, P], bf16, tag="ident")
        nc.scalar.dma_start(ident[:], ident_in[:])
        sb_t = const.tile([P, P], bf16, tag="sb_t")
        nc.scalar.dma_start(sb_t[:], sb_in[:])
        mask0_t = const.tile([P, 1], f32, tag="mask0")
        nc.scalar.dma_start(mask0_t[:], mask0_in[:])

        ones_f = const.tile([1, P], f32, tag="ones_f")
        nc.vector.memset(ones_f[:], 1.0)
        ones_b = const.tile([1, P], bf16, tag="ones_b")
        nc.vector.memset(ones_b[:], 1.0)

        # ---- prologue: broadcast weights / attention rows (f32) ----
        def bcast_f32(row_ap, width):
            bp = tpp.tile([P, IND], f32, tag="tp")
            nc.tensor.matmul(
                bp[:, :width], lhsT=ones_f[:], rhs=row_ap, start=True, stop=True,
                skip_group_check=True,
            )
            return bp

        wsp = bcast_f32(ws_t[:], IND)
        wsb = const.tile([P, IND], bf16, tag="wsb")
        nc.vector.tensor_copy(wsb[:], wsp[:, :IND])
        wsf = const.tile([P, IND], f32, tag="wsf")
        nc.vector.tensor_copy(wsf[:], wsp[:, :IND])
        wdp = bcast_f32(wd_t[:], IND)
        wdb = const.tile([P, IND], f32, tag="wdb")
        nc.vector.tensor_copy(wdb[:], wdp[:, :IND])
        bp = bcast_f32(bias_t[:], OUTD)
        bias_b = const.tile([P, OUTD], f32, tag="bias_b")
        nc.vector.tensor_copy(bias_b[:], bp[:, :OUTD])

        # a_d per local row (bf16 column feeds the per-tile S^T matmuls)
        scr0 = scrp.tile([P, IND], f32, tag="scrf")
        ad_col = smallp.tile([P, 1], f32, tag="ad_col")
        nc.vector.scalar_tensor_tensor(
            out=scr0[:], in0=xself_t[:], scalar=0.0, in1=wdb[:],
            op0=Alu.bypass, op1=Alu.mult, accum_out=ad_col[:],
        )
        ad_bf = const.tile([P, 1], bf16, tag="ad_bf")
        nc.vector.tensor_copy(ad_bf[:], ad_col[:])

        # a_d[0] (node 0), broadcast to all partitions (for the dst==0 block)
        scr1 = scrp.tile([P, IND], f32, tag="scrf")
        ad0_f = smallp.tile([1, 1], f32, tag="ad0_f")
        nc.vector.scalar_tensor_tensor(
            out=scr1[0:1, :], in0=x0_t[:], scalar=0.0, in1=wdb[0:1, :],
            op0=Alu.bypass, op1=Alu.mult, accum_out=ad0_f[:],
        )
        ad0_bf = smallp.tile([1, 1], bf16, tag="ad0_bf")
        nc.vector.tensor_copy(ad0_bf[:], ad0_f[:])
        ad0p = tpp.tile([P, IND], f32, tag="tp")
        nc.tensor.matmul(
            ad0p[:, 0:1], lhsT=ones_b[:], rhs=ad0_bf[:], start=True, stop=True,
            skip_group_check=True,
        )
        ad0_col = smallp.tile([P, 1], f32, tag="ad0_col")
        nc.vector.tensor_copy(ad0_col[:], ad0p[:, 0:1])

        acc_b = accp.tile([P, IND + 1], f32, tag="acc")
        NACC = 3
        acc_m = []
        for _ai in range(NACC):
            acc_mi = accp.tile([P, IND + 1], f32, tag="acc")
            acc_m.append(acc_mi)
        # single persistent ad PSUM tile; groups alternate column halves
        adp_t = adpp.tile([P, 2 * KGRP], f32, tag="adp")

        # ---- epilogue helpers ----
        def out_phase(acc, outp, first, last, tag, eng=None):
            eng = eng or nc.vector
            u_bf = accsb.tile([P, IND], bf16, tag="u_bf")
            eng.tensor_copy(u_bf[:], acc[:, 0:IND])
            z = smallp.tile([P, 1], f32, tag=f"z{tag}")
            eng.tensor_scalar_add(z[:], acc[:, IND : IND + 1], 1e-30)
            for ci in range(2):
                tp = tpp.tile([P, P], bf16, tag="tp")
                nc.tensor.transpose(tp[:], u_bf[:, ci * P : (ci + 1) * P], ident[:])
                uT = utp.tile([P, P], bf16, tag="uT")
                eng.tensor_copy(uT[:], tp[:])
                nc.tensor.matmul(
                    outp[:], lhsT=uT[:], rhs=(W0 if ci == 0 else W1)[:],
                    start=(first and ci == 0), stop=(last and ci == 1),
                    skip_group_check=True,
                )
            return z

        # xo = elu((u@W)/z + bias), for the dst==0 accumulator
        def out_block(acc, tag):
            outp = outpp.tile([P, OUTD], f32, tag="outpb")
            z = out_phase(acc, outp, True, True, tag)
            rz = smallp.tile([P, 1], f32, tag=f"rz{tag}")
            nc.vector.reciprocal(rz[:], z[:])
            outn = xotr.tile([P, OUTD], f32, tag="outn")
            nc.vector.scalar_tensor_tensor(
                out=outn[:], in0=outp[:], scalar=rz[:], in1=bias_b[:],
                op0=Alu.mult, op1=Alu.add,
            )
            tneg = xotr.tile([P, OUTD], f32, tag="tneg")
            nc.scalar.activation(tneg[:], outn[:], Act.Relu, scale=-1.0)
            texp = xotr.tile([P, OUTD], f32, tag="texp")
            nc.scalar.activation(texp[:], tneg[:], Act.Exp, scale=-1.0)
            xo = xop.tile([P, OUTD], bf16, tag="xo")
            nc.vector.scalar_tensor_tensor(
                out=xo[:], in0=texp[:], scalar=-1.0, in1=outn[:],
                op0=Alu.add, op1=Alu.max,
            )
            return xo

        # ---- processing sequence over S-block indices: the self tile
        # first (its direct DMA lands early), then the gathered data tiles
        # in stream order; tile 0 (pos 1) also carries the dst0 edges.
        # seq[pos] = S/ST block index; source AP derived from it. ----
        seq = [T_d] + list(range(T_d))
        NPOS = len(seq)
        SPECIAL = 1  # position of data tile 0 (holds the dst0 edges)

        def pos_src(pos):
            sb = seq[pos]
            if sb == T_d:
                return xself_t[:]
            return xg_d[sb][:]

        # ---- compute groups, software-pipelined with a one-group skew so
        # the in-order engine queues never head-of-line block on a
        # not-yet-gathered tile; the endgame runs single-tile groups so the
        # post-stream dependency chain is as short as possible ----
        groups = []
        rest = list(range(NPOS))
        taper = [2, 1, 1, 1] if len(rest) > KGRP + 6 else []
        head = len(rest) - sum(taper)
        for i in range(0, head, KGRP):
            groups.append(rest[i : min(i + KGRP, head)])
        pos = head
        for tsz in taper:
            groups.append(rest[pos : pos + tsz])
            pos += tsz
        ngroups = len(groups)

        # accumulator index by position: the final acc covers only the
        # endgame tiles so its epilogue chain after the stream is short
        c1 = (2 * NPOS) // 5
        c2 = (3 * NPOS) // 4
        bounds = [0, c1, c2, NPOS]

        def pos_acc(pos):
            for i in range(NACC):
                if bounds[i] <= pos < bounds[i + 1]:
                    return i
            raise AssertionError

        state = {}

        def stage_a(gi):
            poss = groups[gi]
            as_g = grpp.tile([P, KGRP], f32, tag="as")
            a0 = (gi % 2) * KGRP
            adp = adp_t[:, a0 : a0 + KGRP]
            xbfs = []
            for j, pos in enumerate(poss):
                sb = seq[pos]
                src = pos_src(pos)
                nc.tensor.matmul(
                    adp[:, j : j + 1], lhsT=st_t[:, sb * P : (sb + 1) * P],
                    rhs=ad_bf[:], start=True, stop=True, skip_group_check=True,
                )
                # bf16 cast of the gathered rows + constant ones column;
                # alternate engines mid-stream (neither DVE nor ACT may
                # wall); endgame casts stay on DVE so ACT's queue is clear
                # for the leaky/exp/scale chain
                xbf = xbfp.tile([P, W258], bf16, tag="xbf")
                if pos % 3 != 0 or pos >= NPOS - 5:
                    nc.scalar.activation(xbf[:, 0:IND], src, Act.Copy)
                else:
                    nc.vector.tensor_copy(xbf[:, 0:IND], src)
                nc.vector.memset(xbf[:, IND : IND + 1], 1.0)
                if pos >= NPOS - 5:
                    scr = scrp.tile([P, IND], f32, tag="scrf")
                    nc.vector.scalar_tensor_tensor(
                        out=scr[:], in0=src, scalar=0.0, in1=wsf[:],
                        op0=Alu.bypass, op1=Alu.mult,
                        accum_out=as_g[:, j : j + 1],
                    )
                else:
                    scr = scrp.tile([P, IND], bf16, tag="scr")
                    nc.vector.scalar_tensor_tensor(
                        out=scr[:], in0=xbf[:, 0:IND], scalar=0.0, in1=wsb[:],
                        op0=Alu.bypass, op1=Alu.mult,
                        accum_out=as_g[:, j : j + 1],
                    )
                xbfs.append(xbf)
            state[gi] = (as_g, adp, xbfs)

        def stage_b(gi):
            poss = groups[gi]
            k = len(poss)
            as_g, adp, xbfs = state.pop(gi)
            v_g = grpp.tile([P, KGRP], f32, tag="v")
            if SPECIAL in poss:
                # tile 0's dst0 rows get a_d[node 0] (their ST column is
                # all-zero, so the ad matmul left 0 there)
                ad_g = grpp.tile([P, KGRP], f32, tag="adg")
                nc.vector.tensor_copy(ad_g[:, 0:k], adp[:, 0:k])
                js = poss.index(SPECIAL)
                nc.vector.scalar_tensor_tensor(
                    out=ad_g[:, js : js + 1], in0=mask0_t[:],
                    scalar=ad0_col[:], in1=ad_g[:, js : js + 1],
                    op0=Alu.mult, op1=Alu.add,
                )
                nc.vector.tensor_tensor(
                    out=v_g[:, 0:k], in0=as_g[:, 0:k], in1=ad_g[:, 0:k],
                    op=Alu.add,
                )
            else:
                nc.vector.tensor_tensor(
                    out=v_g[:, 0:k], in0=as_g[:, 0:k], in1=adp[:, 0:k],
                    op=Alu.add,
                )
            e_g = grpp.tile([P, KGRP], f32, tag="e")
            nc.vector.scalar_tensor_tensor(
                out=e_g[:, 0:k], in0=v_g[:, 0:k], scalar=NEG_SLOPE,
                in1=v_g[:, 0:k], op0=Alu.mult, op1=Alu.max,
            )
            p_g = grpp.tile([P, KGRP], f32, tag="p")
            nc.scalar.activation(p_g[:, 0:k], e_g[:, 0:k], Act.Exp)
            for j, pos in enumerate(poss):
                sb = seq[pos]
                s_p = spp.tile([P, P], bf16, tag="sp")
                if pos < NPOS - 5:
                    nc.scalar.activation(
                        s_p[:], s_t[:, sb * P : (sb + 1) * P], Act.Copy,
                        scale=p_g[:, j : j + 1],
                    )
                else:
                    nc.vector.tensor_scalar(
                        out=s_p[:], in0=s_t[:, sb * P : (sb + 1) * P],
                        scalar1=p_g[:, j : j + 1], scalar2=None, op0=Alu.mult,
                    )
                rhs = xbfs[j][:, 0 : IND + 1]
                ai = pos_acc(pos)
                nc.tensor.matmul(
                    acc_m[ai][:], lhsT=s_p[:], rhs=rhs,
                    start=(pos == bounds[ai]),
                    stop=(pos == bounds[ai + 1] - 1),
                    skip_group_check=True,
                )
                if pos == SPECIAL:
                    # dst0 edges: scatter tile 0's reserved rows into the
                    # shared dst0 accumulator (single-matmul group)
                    s_pb = spp.tile([P, P], bf16, tag="spb")
                    nc.scalar.activation(
                        s_pb[:], sb_t[:], Act.Copy, scale=p_g[:, j : j + 1],
                    )
                    nc.tensor.matmul(
                        acc_b[:], lhsT=s_pb[:], rhs=rhs,
                        start=True, stop=True, skip_group_check=True,
                    )
            if SPECIAL in poss:
                # dst==0 block complete: fold its epilogue under the stream
                state["xo_b"] = out_block(acc_b, "b")

        # group index right after which accumulator ai is complete
        def acc_done_group(ai):
            last_pos = bounds[ai + 1] - 1
            for gi, poss in enumerate(groups):
                if last_pos in poss:
                    return gi
            raise AssertionError

        done_at = {acc_done_group(ai): ai for ai in range(NACC - 1)}

        outp_m = outpp.tile([P, OUTD], f32, tag="outp")
        zs = []
        for gi in range(ngroups):
            stage_a(gi)
            stage_b(gi)
            if gi in done_at:
                ai = done_at[gi]
                zs.append(out_phase(acc_m[ai], outp_m, ai == 0, False, f"m{ai}"))
        xo_b = state["xo_b"]
        zz01 = smallp.tile([P, 1], f32, tag="zz01")
        nc.vector.tensor_tensor(out=zz01[:], in0=zs[0][:], in1=zs[1][:], op=Alu.add)

        zs.append(out_phase(acc_m[NACC - 1], outp_m, False, True, f"m{NACC-1}"))
        zz = smallp.tile([P, 1], f32, tag="zz")
        nc.vector.tensor_tensor(out=zz[:], in0=zz01[:], in1=zs[2][:], op=Alu.add)
        rz = smallp.tile([P, 1], f32, tag="rzm")
        nc.vector.reciprocal(rz[:], zz[:])
        outn = xotr.tile([P, OUTD], f32, tag="outn")
        nc.vector.scalar_tensor_tensor(
            out=outn[:], in0=outp_m[:], scalar=rz[:], in1=bias_b[:],
            op0=Alu.mult, op1=Alu.add,
        )
        tneg = xotr.tile([P, OUTD], f32, tag="tneg")
        nc.scalar.activation(tneg[:], outn[:], Act.Relu, scale=-1.0)
        texp = xotr.tile([P, OUTD], f32, tag="texp")
        nc.scalar.activation(texp[:], tneg[:], Act.Exp, scale=-1.0)
        xo_m = xop.tile([P, OUTD], bf16, tag="xo")
        nc.vector.scalar_tensor_tensor(
            out=xo_m[:], in0=texp[:], scalar=-1.0, in1=outn[:],
            op0=Alu.add, op1=Alu.max,
        )

        # ---- y = elu(dot(xo[0], xo_m[j])) ----
        xo0p = tpp.tile([P, IND], f32, tag="tp")
        nc.tensor.matmul(
            xo0p[:, :OUTD], lhsT=ones_b[:], rhs=xo_b[0:1, :], start=True, stop=True,
            skip_group_check=True,
        )
        xo0s = const.tile([P, OUTD], bf16, tag="xo0s")
        nc.vector.tensor_copy(xo0s[:], xo0p[:, :OUTD])
        dscr = scrp.tile([P, OUTD], f32, tag="dscr")
        d_sb = smallp.tile([P, 1], f32, tag="d_sb")
        nc.vector.scalar_tensor_tensor(
            out=dscr[:], in0=xo_m[:], scalar=0.0, in1=xo0s[:],
            op0=Alu.bypass, op1=Alu.mult, accum_out=d_sb[:],
        )
        yneg = smallp.tile([P, 1], f32, tag="yneg")
        nc.scalar.activation(yneg[:], d_sb[:], Act.Relu, scale=-1.0)
        yexp = smallp.tile([P, 1], f32, tag="yexp")
        nc.scalar.activation(yexp[:], yneg[:], Act.Exp, scale=-1.0)
        y_bf = smallp.tile([P, 1], bf16, tag="y_bf")
        nc.vector.scalar_tensor_tensor(
            out=y_bf[:], in0=yexp[:], scalar=-1.0, in1=d_sb[:],
            op0=Alu.add, op1=Alu.max,
        )
        # write y as a contiguous [1, P] row (column DMA has a huge
        # HBM completion delay that the kernel-tail barrier waits out)
        yrp = tpp.tile([P, P], bf16, tag="tp")
        nc.tensor.transpose(yrp[:1, :], y_bf[:], ident[:])
        y_row = smallp.tile([1, P], f32, tag="y_row")
        nc.vector.tensor_copy(y_row[:], yrp[:1, :P])
        nc.sync.dma_start(y_out[:], y_row[:])

    nc.compile()
    return nc


def _get_program(n_nodes, T_d):
    key = (n_nodes, T_d)
    if key not in _CACHE:
        _CACHE[key] = _build_program(n_nodes, T_d)
    return _CACHE[key]


def _pack_cols(vals, T, pad, dtype):
    """[n] -> [P, T] column-per-tile layout (tile t, lane p) = vals[t*P+p]."""
    npad = T * P - len(vals)
    v = np.concatenate([vals, np.full(npad, pad, vals.dtype)])
    return np.ascontiguousarray(v.reshape(T, P).T).astype(dtype)


def _onehot_blocks(dst_cols):
    """dst_cols [P, T] -> (S [P, T*P], ST [P, T*P]) one-hot bf16 blocks.
    S_t[e, j] = (dst[e, t] == j); ST_t = S_t^T. dst==P rows are all-zero."""
    Pn = P
    T = dst_cols.shape[1]
    S = np.zeros((Pn, T * Pn), dtype=BF16)
    ST = np.zeros((Pn, T * Pn), dtype=BF16)
    e_idx, t_idx = np.nonzero(dst_cols < Pn)
    j_idx = dst_cols[e_idx, t_idx]
    S[e_idx, t_idx * Pn + j_idx] = 1
    ST[j_idx, t_idx * Pn + e_idx] = 1
    return np.ascontiguousarray(S), np.ascontiguousarray(ST)


def prepare(x, edge_index, W, att_src, att_dst, bias, item_len):
    """Python-side edge partitioning; returns (nc, in_maps, item_len)."""
    item_len = int(np.asarray(item_len))
    x = np.ascontiguousarray(np.asarray(x, np.float32))
    W = np.ascontiguousarray(np.asarray(W, np.float32))
    att_src = np.asarray(att_src, np.float32)
    att_dst = np.asarray(att_dst, np.float32)
    bias = np.asarray(bias, np.float32)
    n_nodes = x.shape[0]
    assert x.shape[1] == IND and W.shape == (IND, OUTD)
    assert item_len <= N_CORES * P, "kernel supports item_len <= 1024"

    src = np.asarray(edge_index[0])
    dst = np.asarray(edge_index[1])
    keep = dst < item_len
    src_f = src[keep].astype(np.int32)
    dst_f = dst[keep].astype(np.int32)

    # dst==0 edges (graph edges + node-0 self loop), shared by all cores;
    # they ride in the reserved tail slots of every core's data tile 0
    sel0 = dst_f == 0
    b_src = np.concatenate([src_f[sel0], np.zeros(1, np.int32)])
    nb0 = len(b_src)
    assert nb0 <= P, "dst0 fan-in exceeds one tile"

    blk = dst_f // P
    order = np.argsort(blk, kind="stable")
    src_f = src_f[order]
    dst_f = dst_f[order]
    blk = blk[order]
    bounds = np.searchsorted(blk, np.arange(N_CORES + 1))
    max_edges = max(int(bounds[k + 1] - bounds[k]) for k in range(N_CORES))
    T_d = max(1, math.ceil((max_edges + nb0) / P))
    nslots = T_d * P

    nc = _get_program(n_nodes, T_d)

    # host weight preprocessing
    ws_r = np.ascontiguousarray((W @ att_src).astype(np.float32).reshape(1, IND))
    wd_r = np.ascontiguousarray((W @ att_dst).astype(np.float32).reshape(1, IND))
    w_bf = np.ascontiguousarray(W.astype(BF16))
    ident = np.eye(P, dtype=np.float32).astype(BF16)
    x0 = np.ascontiguousarray(x[0:1])
    bias_r = np.ascontiguousarray(bias.reshape(1, OUTD))

    # reserved slots: tail of tile 0 (slot = tile-major index t*P + p)
    reserved = np.arange(P - nb0, P)
    data_slots = np.concatenate([np.arange(0, P - nb0), np.arange(P, nslots)])
    S_b = np.zeros((P, P), dtype=BF16)
    S_b[P - nb0 :, 0] = 1
    mask0 = np.zeros((P, 1), np.float32)
    mask0[P - nb0 :, 0] = 1.0
    T_idx = max(T_d, 128)

    in_maps = []
    for k in range(N_CORES):
        lo, hi = bounds[k], bounds[k + 1]
        es = src_f[lo:hi]
        ed = dst_f[lo:hi] - k * P
        src_all = np.zeros(nslots, np.int32)
        dst_all_f = np.full(nslots, P, np.int32)
        dslots = data_slots[: len(es)]
        src_all[dslots] = es
        dst_all_f[dslots] = ed
        src_all[reserved] = b_src
        eidx = np.ascontiguousarray(src_all.reshape(T_d, P).T)
        idx_pad = np.zeros((P, T_idx - T_d), np.int32)
        dst_cols = [np.ascontiguousarray(dst_all_f.reshape(T_d, P).T)]
        self_dst = np.arange(P, dtype=np.int32)
        if (k + 1) * P > item_len:
            self_dst = np.where(
                np.arange(k * P, (k + 1) * P) < item_len, self_dst, P
            ).astype(np.int32)
        dst_cols.append(self_dst[:, None])
        dst_all = np.concatenate(dst_cols, axis=1)
        S, ST = _onehot_blocks(dst_all)
        xself = np.ascontiguousarray(
            x[np.minimum(np.arange(k * P, (k + 1) * P), n_nodes - 1)]
        )
        m = {
            "x_in": x,
            "idx_in": np.ascontiguousarray(np.concatenate([eidx, idx_pad], axis=1)),
            "xself_in": xself,
            "x0_in": x0,
            "s_in": S,
            "st_in": ST,
            "ws_in": ws_r,
            "wd_in": wd_r,
            "w_in": w_bf,
            "bias_in": bias_r,
            "ident_in": ident,
            "sb_in": S_b,
            "mask0_in": mask0,
        }
        in_maps.append(m)
    return nc, in_maps, item_len


def assemble(results, item_len):
    y_all = np.concatenate(
        [np.asarray(results[k]["y_out"], np.float32).ravel() for k in range(N_CORES)]
    )
    return y_all[1:item_len].astype(np.float32)


def kernel(x, edge_index, W, att_src, att_dst, bias, item_len):
    from concourse import bass_utils

    nc, in_maps, item_len = prepare(
        x, edge_index, W, att_src, att_dst, bias, item_len
    )
    res = bass_utils.run_bass_kernel_spmd(nc, in_maps, core_ids=list(range(N_CORES)))
    return assemble(res.results, item_len)
